# revision 20
# baseline (speedup 1.0000x reference)
"""GAT (4-layer, 8-head) message-passing kernel for 8 Trainium2 NeuronCores.

Strategy (self-contained; shapes hardcoded from the problem spec):
  - Nodes are sharded by destination across 8 cores (6250 nodes each);
    within each core nodes are permuted by (degree, lo-hi balance) so that
    128-node tiles have near-uniform in-degree.
  - Per layer: each core computes hW = f @ W for its local nodes (PE),
    per-node attention coefficients asrc/adst (DVE), packs [bf16 h | f32
    asrc] into 256-byte table rows, and AllGathers the table to all cores.
  - Per-edge work uses dma_gather (256B rows) with int16 indices. The
    int16 range limit is handled with two gather bases (row 0 and row
    3*SHARD); sources from ranks 3-4 are reachable from either base, which
    lets per-node slot assignment meet uniform per-tile column cuts with
    only a narrow double-gathered strip.
  - The appended self-loop of every node is NOT gathered: slot column 0 is
    filled by an on-chip copy of the local staging tile.
  - Slots land [node-on-partition, column]; softmax (no max-subtraction
    needed, logits are O(10)) via ACT Exp; the segment sum (numerator and
    denominator in one shot) is identity-lhsT PE matmuls accumulating
    columns into PSUM.
  - The dense phase of layer l+1 is fused into the edge loop of layer l so
    PE/DVE work hides under the gather descriptor generation (GpSimd).
  - global_mean_pool: per-core onehot matmul accumulation + AllReduce,
    then the final 32->2 linear on every core (identical outputs).
"""
import sys
sys.path.insert(0, "/opt/trn_rl_repo")

import numpy as np
import ml_dtypes

BF16 = ml_dtypes.bfloat16

N = 50000
E = 1600000
G_GRAPHS = 128
F_IN = 128
HEADS, HID = 8, 8
C_HID = 64
OUT_C = 32
NC = 8
NLOC = N // NC            # 6250
SHARD = NLOC + 2          # 6252 rows per rank in the table (+Z +NEG)
NTILES = 49
NPAD = NTILES * 128
BASE1 = 7 * SHARD + NLOC + 1 - 32767   # 17248: widest dual-reach zone
NEG_VAL = -1e30
Z_OFF, NEG_OFF = NLOC, NLOC + 1                    # base0-relative (rank 0)
Z_OFF1 = 3 * SHARD + NLOC - BASE1                  # base1-relative (rank 3)
NEG_OFF1 = Z_OFF1 + 1


# ----------------------------------------------------------------------
# CPU preprocessing
# ----------------------------------------------------------------------

def _preprocess(edge_index, batch):
    import time as _time
    # No self-loop append: the appended loops are handled on-chip (col 0).
    src = np.asarray(edge_index[0]).astype(np.int64)
    dst = np.asarray(edge_index[1]).astype(np.int64)
    batch = np.asarray(batch).astype(np.int64)
    deg = np.bincount(dst, minlength=N)
    owner = np.repeat(np.arange(NC), NLOC)

    # classes are row-granular: lo = base0-only, mid = dual-reach, hi = base1-
    # only. Rows depend on positions, so seed with a deg-only sort, derive
    # classes, then re-sort with the balance secondary key.
    order_pos = np.empty(N, np.int64)
    perm = []
    for c in range(NC):
        local = np.arange(c * NLOC, (c + 1) * NLOC)
        p = local[np.argsort(-deg[local], kind="stable")]
        perm.append(p)
        order_pos[p] = np.arange(NLOC)

    def node_classes():
        row = owner * SHARD + order_pos
        return np.where(row[src] < BASE1, 0,
                        np.where(row[src] <= 32767, 1, 2))

    cls = node_classes()
    nlow_of = np.bincount(dst, weights=(cls == 0), minlength=N).astype(np.int64)
    nhigh_of = np.bincount(dst, weights=(cls == 2), minlength=N).astype(np.int64)
    for c in range(NC):
        local = np.arange(c * NLOC, (c + 1) * NLOC)
        key = deg[local] * 10000 + (nlow_of[local] - nhigh_of[local]) * 30
        p = local[np.argsort(-key, kind="stable")]
        perm[c] = p
        order_pos[p] = np.arange(NLOC)
    cls = node_classes()
    nlow_of = np.bincount(dst, weights=(cls == 0), minlength=N).astype(np.int64)
    nhigh_of = np.bincount(dst, weights=(cls == 2), minlength=N).astype(np.int64)

    # Local search: within-core swaps reducing the global per-tile objective
    # sum(Dmax + max(0, Lmax+Hmax-Dmax)). Swaps in ranks 2/5 must not cross
    # the class row boundaries (classes above are frozen).
    bound2 = BASE1 - 2 * SHARD
    bound5 = 32767 - 5 * SHARD
    t0 = _time.time()

    def tile_stats(t):
        s, e = t * 128, min(t * 128 + 128, NLOC)
        dm = lm = hm = 0
        for c in range(NC):
            p = perm[c][s:e]
            dm = max(dm, int(deg[p].max()))
            lm = max(lm, int(nlow_of[p].max()))
            hm = max(hm, int(nhigh_of[p].max()))
        return [dm, lm, hm]

    def tcost(st):
        return st[0] + max(0, st[1] + st[2] - st[0])

    stats = [tile_stats(t) for t in range(NTILES)]
    for _pass in range(6):
        improved = False
        for t in range(NTILES):
            s, e = t * 128, min(t * 128 + 128, NLOC)
            t2range = (range(max(0, t - 4), min(NTILES, t + 5))
                       if _pass < 4 else range(NTILES))
            for t2 in t2range:
                if t2 == t:
                    continue
                s2, e2 = t2 * 128, min(t2 * 128 + 128, NLOC)
                c0 = tcost(stats[t]) + tcost(stats[t2])
                for c in range(NC):
                    p = perm[c]
                    win, win2 = p[s:e], p[s2:e2]
                    for arr in (nlow_of, nhigh_of, deg):
                        i = s + int(np.argmax(arr[win]))
                        for j in (s2 + int(np.argmin(arr[win2])),
                                  s2 + int(np.argmax(deg[win2]))):
                            if c == 2 and (i >= bound2) != (j >= bound2):
                                continue
                            if c == 5 and (i <= bound5) != (j <= bound5):
                                continue
                            p[i], p[j] = p[j], p[i]
                            st1, st2 = tile_stats(t), tile_stats(t2)
                            if tcost(st1) + tcost(st2) < c0:
                                stats[t], stats[t2] = st1, st2
                                c0 = tcost(st1) + tcost(st2)
                                improved = True
                            else:
                                p[i], p[j] = p[j], p[i]
        if not improved or _time.time() - t0 > 35:
            break
    for c in range(NC):
        order_pos[perm[c]] = np.arange(NLOC)
    row_of = np.empty(N, np.int64)
    for c in range(NC):
        row_of[perm[c]] = c * SHARD + order_pos[perm[c]]

    core_e = dst // NLOC
    pos_e = order_pos[dst]
    srow = row_of[src]

    ndeg = np.zeros((NC, NPAD), np.int32)
    nlow = np.zeros((NC, NPAD), np.int32)
    nmid = np.zeros((NC, NPAD), np.int32)
    np.add.at(ndeg, (core_e, pos_e), 1)
    np.add.at(nlow, (core_e, pos_e), (cls == 0).astype(np.int32))
    np.add.at(nmid, (core_e, pos_e), (cls == 1).astype(np.int32))
    nhigh = ndeg - nlow - nmid

    # Per-tile gather geometry: D = max degree (gathered cols, excl. self),
    # K = base0-only col count, W = double-gathered strip width.
    D = np.zeros(NTILES, np.int32)
    K = np.zeros(NTILES, np.int32)
    Wd = np.zeros(NTILES, np.int32)
    for t in range(NTILES):
        s = t * 128
        real = min(128, NLOC - s) if s < NLOC else 0
        if real:
            dm = int(ndeg[:, s:s + real].max())
            Lmax = int(nlow[:, s:s + real].max())
            Hmax = int(nhigh[:, s:s + real].max())
        else:
            dm = Lmax = Hmax = 0
        w = max(0, Lmax + Hmax - dm)
        k = dm - Hmax
        D[t], K[t], Wd[t] = dm, k, w

    # Slot assignment: lo+mid left-fill from slot 0, hi right-aligned at D.
    key2 = ((core_e * NPAD + pos_e) * 4 + cls)
    o = np.argsort(key2, kind="stable")
    srow_s, core_s, pos_s, cls_s = srow[o], core_e[o], pos_e[o], cls[o]
    gid = core_s * NPAD + pos_s
    first = np.r_[True, gid[1:] != gid[:-1]]
    cum = np.arange(len(gid)) - np.maximum.accumulate(
        np.where(first, np.arange(len(gid)), 0))
    t_of = pos_s // 128
    col = np.where(cls_s < 2, cum,
                   D[t_of] - nhigh[core_s, pos_s] + cum
                   - nlow[core_s, pos_s] - nmid[core_s, pos_s])
    Dmax = int(D.max())
    which = np.full((NC, NPAD, Dmax), 4, np.int8)   # 4 = pad
    rowg = np.zeros((NC, NPAD, Dmax), np.int64)
    which[core_s, pos_s, col] = cls_s
    rowg[core_s, pos_s, col] = srow_s

    batchrel = np.full((NC, NPAD), 255, np.int64)
    for c in range(NC):
        batchrel[c, :NLOC] = batch[perm[c]]
    cnt = np.bincount(batch, minlength=G_GRAPHS).astype(np.float32)
    return dict(perm=perm, which=which, rowg=rowg, D=D, K=K, Wd=Wd,
                batchrel=batchrel, cnt=cnt)


def _build_idx_core(pp, c):
    """Linear int16 idx stream for core c: per tile, calls P0=[X|A], P1=[Y|B]."""
    chunks = []
    for t in range(NTILES):
        d, k, w = int(pp["D"][t]), int(pp["K"][t]), int(pp["Wd"][t])
        wh = pp["which"][c, t * 128:(t + 1) * 128, :d]
        rg = pp["rowg"][c, t * 128:(t + 1) * 128, :d]
        rel0 = rg
        rel1 = rg - BASE1
        # P0 from base0: X cols [0,k): lo/mid -> row, pad -> NEG
        #                A cols [k,k+w): lo/mid -> row, hi -> Z, pad -> NEG
        wx = wh[:, :k]
        iX = np.where(wx <= 1, rel0[:, :k], NEG_OFF)      # cls2 impossible in X
        wa = wh[:, k:k + w]
        iA = np.where(wa <= 1, rel0[:, k:k + w],
                      np.where(wa == 2, Z_OFF, NEG_OFF))
        # P1 from base1: Y cols [k+w,d): mid/hi -> row-BASE1, pad -> NEG
        #                B cols [k,k+w): hi -> row-BASE1, else -> Z
        wy = wh[:, k + w:]
        iY = np.where((wy == 1) | (wy == 2), rel1[:, k + w:], NEG_OFF1)
        iB = np.where(wa == 2, rel1[:, k:k + w], Z_OFF1)
        for arr in (iX, iA, iY, iB):
            # linearize: for col j, node i -> stream slot j*128+i
            chunks.append(arr.T.reshape(-1))
    lin = np.concatenate(chunks).astype(np.int16)
    assert lin.min() >= 0
    return lin.reshape(-1, 16).T.copy()    # [16, TOT/16]


def _idx_offsets(pp):
    """Column offsets (in idx units /16) of each call, per tile."""
    offs = []
    cur = 0
    for t in range(NTILES):
        d, k, w = int(pp["D"][t]), int(pp["K"][t]), int(pp["Wd"][t])
        o = {}
        for name, cols in (("P0", k + w), ("P1", d - k)):
            o[name] = (cur // 16, cols * 128)
            cur += cols * 128
        offs.append(o)
    return offs, cur


# ----------------------------------------------------------------------
# Bass program
# ----------------------------------------------------------------------

def _build_program(pp, tot_idx):
    import concourse.bacc as bacc
    import concourse.mybir as mybir
    import concourse.tile as tile
    from concourse.library_config import mlp

    dt = mybir.dt
    AF = mybir.ActivationFunctionType
    ALU = mybir.AluOpType

    D, K, Wd = pp["D"], pp["K"], pp["Wd"]
    Dmax = int(D.max())
    Wmax = int(Wd.max())
    offs, _ = _idx_offsets(pp)

    nc = bacc.Bacc("TRN2", target_bir_lowering=False, debug=False,
                   num_devices=NC)

    t_x = nc.dram_tensor("x", [NPAD, F_IN], dt.bfloat16, kind="ExternalInput")
    t_idx = nc.dram_tensor("idx", [16, tot_idx // 16], dt.int16,
                           kind="ExternalInput")
    t_w = [nc.dram_tensor(f"W{l}", [F_IN if l == 0 else C_HID,
                                    OUT_C if l == 3 else C_HID],
                          dt.float32, kind="ExternalInput") for l in range(4)]
    # packed broadcast row: per layer AS|AD|B (Cl each), then bl (2)
    w_offs = {}
    cur = 0
    for l in range(4):
        Cl = OUT_C if l == 3 else C_HID
        w_offs[f"AS{l}"] = cur; cur += Cl
        w_offs[f"AD{l}"] = cur; cur += Cl
        w_offs[f"B{l}"] = cur; cur += Cl
    w_offs["bl"] = cur; cur += 2
    WROW = cur                                           # 674
    t_wrow = nc.dram_tensor("wrow", [1, WROW], dt.float32,
                            kind="ExternalInput")
    t_identb = nc.dram_tensor("identb", [128, 128], dt.bfloat16,
                              kind="ExternalInput")
    t_identf = nc.dram_tensor("identf", [128, 128], dt.float32,
                              kind="ExternalInput")
    t_iotag = nc.dram_tensor("iotag", [128, 128], dt.float32,
                             kind="ExternalInput")
    t_brel = nc.dram_tensor("brel", [128, NTILES], dt.float32,
                            kind="ExternalInput")
    t_zneg = nc.dram_tensor("zneg", [2, 128], dt.bfloat16,
                            kind="ExternalInput")
    t_invc = nc.dram_tensor("invc", [128, 1], dt.float32,
                            kind="ExternalInput")
    t_wl = nc.dram_tensor("Wl", [OUT_C, 2], dt.float32, kind="ExternalInput")
    t_out = nc.dram_tensor("out", [G_GRAPHS, 2], dt.float32,
                           kind="ExternalOutput")

    with tile.TileContext(nc) as tc:
        with tc.tile_pool(name="res", bufs=1) as res, \
             tc.tile_pool(name="work", bufs=2) as work, \
             tc.tile_pool(name="wk3", bufs=3) as wk3, \
             tc.tile_pool(name="ps", bufs=2, space="PSUM") as ps, \
             tc.tile_pool(name="pspool", bufs=1, space="PSUM") as pspool, \
             tc.tile_pool(name="dram", bufs=1, space="DRAM") as dram:

            nc.gpsimd.load_library(mlp)

            # ---- resident loads ----
            idx_sb = res.tile([128, tot_idx // 16], dt.int16)
            for g in range(8):
                nc.sync.dma_start(idx_sb[16 * g:16 * g + 16, :], t_idx.ap())
            identb = res.tile([128, 128], dt.bfloat16)
            nc.sync.dma_start(identb[:], t_identb.ap())
            identf = res.tile([128, 128], dt.float32)
            nc.sync.dma_start(identf[:], t_identf.ap())
            iotag = res.tile([128, 128], dt.float32)
            nc.sync.dma_start(iotag[:], t_iotag.ap())
            brel = res.tile([128, NTILES], dt.float32)
            nc.sync.dma_start(brel[:], t_brel.ap())
            zneg = res.tile([2, 128], dt.bfloat16)
            nc.sync.dma_start(zneg[:], t_zneg.ap())
            invc = res.tile([128, 1], dt.float32)
            nc.sync.dma_start(invc[:], t_invc.ap())
            wl_sb = res.tile([OUT_C, 2], dt.float32)
            nc.sync.dma_start(wl_sb[:], t_wl.ap())
            w_sb = []
            for l in range(4):
                Fl = F_IN if l == 0 else C_HID
                Cl = OUT_C if l == 3 else C_HID
                w_sb.append(res.tile([Fl, Cl], dt.float32, tag=f"w{l}",
                                     name=f"w{l}"))
                nc.sync.dma_start(w_sb[l][:], t_w[l].ap())

            # broadcast the packed weight row to all 128 partitions via PE
            wrow_sb = res.tile([1, WROW], dt.float32)
            nc.sync.dma_start(wrow_sb[:], t_wrow.ap())
            ones_sb = res.tile([1, 128], dt.float32)
            nc.vector.memset(ones_sb[:], 1.0)
            bcast = res.tile([128, WROW], dt.float32)
            half = 288
            for o0 in range(0, WROW, half):
                o1 = min(o0 + half, WROW)
                psB = ps.tile([128, half], dt.float32, tag="psA")
                nc.tensor.matmul(psB[:, 0:o1 - o0], ones_sb[:],
                                 wrow_sb[:, o0:o1], start=True, stop=True)
                nc.vector.tensor_copy(bcast[:, o0:o1], psB[:, 0:o1 - o0])

            def as_ap(l):
                Cl = OUT_C if l == 3 else C_HID
                return bcast[:, w_offs[f"AS{l}"]:w_offs[f"AS{l}"] + Cl]

            def ad_ap(l):
                Cl = OUT_C if l == 3 else C_HID
                return bcast[:, w_offs[f"AD{l}"]:w_offs[f"AD{l}"] + Cl]

            def b_ap(l):
                Cl = OUT_C if l == 3 else C_HID
                return bcast[:, w_offs[f"B{l}"]:w_offs[f"B{l}"] + Cl]

            bl_ap = bcast[:, w_offs["bl"]:w_offs["bl"] + 2]

            f_sb = res.tile([128, NTILES, C_HID], dt.float32)
            # ping-pong staging copies (current layer / next layer)
            stg_sb = [res.tile([128, NTILES, 128], dt.bfloat16, tag=f"stg{i}",
                               name=f"stg{i}") for i in range(2)]
            adst_l = [res.tile([128, NTILES, HEADS], dt.float32,
                               tag=f"adst{i}", name=f"adst{i}")
                      for i in range(2)]
            staging = dram.tile([SHARD, 128], dt.bfloat16)

            pool_in = dram.tile([G_GRAPHS, OUT_C], dt.float32)
            pool_out = dram.tile([G_GRAPHS, OUT_C], dt.float32,
                                 addr_space="Shared")
            ps_pool = pspool.tile([128, OUT_C], dt.float32)

            G = [dram.tile([NC * SHARD, 128], dt.bfloat16,
                           addr_space="Shared", tag=f"G{l}", name=f"G{l}")
                 for l in range(4)]

            def dense_tile(l, t, fin):
                """Compute layer-l hW/asrc/adst for tile t from features fin;
                write staging row + stg_sb/adst_l for layer l."""
                Cl = OUT_C if l == 3 else C_HID
                Hl = 1 if l == 3 else HEADS
                AOFF = Cl // 2
                buf = l % 2
                Fl = F_IN if l == 0 else C_HID
                fT = work.tile([F_IN, 128], dt.float32, tag="fT")
                psT = ps.tile([F_IN, 128], dt.float32, tag="psT")
                if l == 0:
                    psTb = psT[:].bitcast(dt.bfloat16)[:, 0:128]
                    nc.tensor.transpose(psTb, fin, identb[:])
                    nc.scalar.activation(fT[:], psTb, AF.Copy)
                else:
                    nc.tensor.transpose(psT[0:C_HID, :], fin, identf[:])
                    nc.scalar.activation(fT[0:Fl, :], psT[0:Fl, :], AF.Copy)
                psH = ps.tile([128, Cl], dt.float32, tag="psH")
                nc.tensor.matmul(psH[:], fT[0:Fl, :], w_sb[l][:],
                                 start=True, stop=True)
                hw = work.tile([128, C_HID], dt.float32, tag="hw")
                nc.vector.tensor_copy(hw[:, 0:Cl], psH[:])

                tmp = work.tile([128, C_HID], dt.float32, tag="tmp")
                nc.vector.tensor_tensor(tmp[:, 0:Cl], hw[:, 0:Cl], as_ap(l),
                                        ALU.mult)
                asrc_t = work.tile([128, HEADS], dt.float32, tag="asrc_t")
                nc.vector.tensor_reduce(
                    asrc_t[:, 0:Hl],
                    tmp[:, 0:Cl].rearrange("p (h c) -> p h c", h=Hl),
                    mybir.AxisListType.X, ALU.add)
                nc.vector.tensor_tensor(tmp[:, 0:Cl], hw[:, 0:Cl], ad_ap(l),
                                        ALU.mult)
                nc.vector.tensor_reduce(
                    adst_l[buf][:, t, 0:Hl],
                    tmp[:, 0:Cl].rearrange("p (h c) -> p h c", h=Hl),
                    mybir.AxisListType.X, ALU.add)

                stg = work.tile([128, 128], dt.bfloat16, tag="stg")
                nc.vector.memset(stg[:, Cl:128], 0)
                nc.scalar.activation(stg[:, 0:Cl], hw[:, 0:Cl], AF.Copy)
                stgf = stg[:].bitcast(dt.float32)
                nc.vector.tensor_copy(stgf[:, AOFF:AOFF + Hl],
                                      asrc_t[:, 0:Hl])
                # persist for the self-loop slot + ship to DRAM for AllGather
                nc.vector.tensor_copy(stg_sb[buf][:, t, :], stg[:])
                rows = min(128, NLOC - t * 128)
                nc.sync.dma_start(
                    staging[t * 128:t * 128 + rows, :], stg[0:rows, :])

            # ---------- layer-0 dense (batched: wide vector ops) ----------
            ft_all = res.tile([128, NTILES, F_IN], dt.bfloat16)
            for t in range(NTILES):
                nc.sync.dma_start(ft_all[:, t, :],
                                  t_x.ap()[t * 128:(t + 1) * 128, :])
            hw_all = res.tile([128, NTILES, C_HID], dt.float32)
            for t in range(NTILES):
                psT0 = ps.tile([F_IN, 128], dt.float32, tag="psT")
                psTb = psT0[:].bitcast(dt.bfloat16)[:, 0:128]
                nc.tensor.transpose(psTb, ft_all[:, t, :], identb[:])
                fT0 = work.tile([F_IN, 128], dt.float32, tag="fT")
                nc.scalar.activation(fT0[:], psTb, AF.Copy)
                psH0 = ps.tile([128, C_HID], dt.float32, tag="psH")
                nc.tensor.matmul(psH0[:], fT0[:], w_sb[0][:],
                                 start=True, stop=True)
                nc.vector.tensor_copy(hw_all[:, t, :], psH0[:])
            tmp_all = res.tile([128, NTILES, C_HID], dt.float32)
            nc.vector.tensor_tensor(
                tmp_all[:], hw_all[:],
                as_ap(0).unsqueeze(1).to_broadcast((128, NTILES, C_HID)),
                ALU.mult)
            asrc_all = res.tile([128, NTILES, HEADS], dt.float32)
            nc.vector.tensor_reduce(
                asrc_all[:],
                tmp_all[:].rearrange("p t (h c) -> p t h c", h=HEADS),
                mybir.AxisListType.X, ALU.add)
            nc.vector.tensor_tensor(
                tmp_all[:], hw_all[:],
                ad_ap(0).unsqueeze(1).to_broadcast((128, NTILES, C_HID)),
                ALU.mult)
            nc.vector.tensor_reduce(
                adst_l[0][:],
                tmp_all[:].rearrange("p t (h c) -> p t h c", h=HEADS),
                mybir.AxisListType.X, ALU.add)
            nc.vector.memset(stg_sb[0][:], 0)
            nc.scalar.activation(stg_sb[0][:, :, 0:C_HID], hw_all[:], AF.Copy)
            stgf_all = stg_sb[0][:].bitcast(dt.float32)
            nc.vector.tensor_copy(stgf_all[:, :, 32:40], asrc_all[:])
            for t in range(NTILES):
                rows = min(128, NLOC - t * 128)
                nc.sync.dma_start(staging[t * 128:t * 128 + rows, :],
                                  stg_sb[0][0:rows, t, :])
            nc.sync.dma_start(staging[NLOC:NLOC + 2, :], zneg[:])
            tc.strict_bb_all_engine_barrier()
            nc.gpsimd.collective_compute(
                "AllGather", mybir.AluOpType.bypass,
                replica_groups=[list(range(NC))],
                ins=[staging.opt()], outs=[G[0].opt()])
            tc.strict_bb_all_engine_barrier()

            # ---------- per-layer edge phase (dense of l+1 fused in) ----------
            for l in range(4):
                Cl = OUT_C if l == 3 else C_HID
                Hl = 1 if l == 3 else HEADS
                hidl = OUT_C if l == 3 else HID
                Wm = Cl + Hl
                AOFF = Cl // 2
                buf = l % 2
                Gap = G[l][:]
                G1ap = G[l][:][BASE1:NC * SHARD, :]

                for t in range(NTILES):
                    d, k, w = int(D[t]), int(K[t]), int(Wd[t])
                    dtot = 1 + d
                    hg = wk3.tile([128, 1 + Dmax + Wmax, 128], dt.bfloat16,
                                  tag="hg")
                    # self-loop slot: on-chip copy of the local staging tile
                    nc.scalar.activation(hg[:, 0, :], stg_sb[buf][:, t, :],
                                         AF.Copy)
                    o = offs[t]
                    if k + w:
                        c0, n = o["P0"]
                        nc.gpsimd.dma_gather(
                            hg[:, 1:1 + k + w, :], Gap,
                            idx_sb[:, c0:c0 + n // 16], n, n, 128,
                            single_packet=False)
                    if d - k:
                        c0, n = o["P1"]
                        nc.gpsimd.dma_gather(
                            hg[:, 1 + k + w:1 + d + w, :], G1ap,
                            idx_sb[:, c0:c0 + n // 16], n, n, 128,
                            single_packet=False)
                    if w:
                        nc.vector.tensor_tensor(
                            hg[:, 1 + k:1 + k + w, 0:Cl],
                            hg[:, 1 + k:1 + k + w, 0:Cl],
                            hg[:, 1 + d:1 + d + w, 0:Cl], ALU.add)
                        hgf = hg[:].bitcast(dt.float32)
                        nc.vector.tensor_tensor(
                            hgf[:, 1 + k:1 + k + w, AOFF:AOFF + Hl],
                            hgf[:, 1 + k:1 + k + w, AOFF:AOFF + Hl],
                            hgf[:, 1 + d:1 + d + w, AOFF:AOFF + Hl], ALU.add)

                    hgf = hg[:].bitcast(dt.float32)
                    e1 = work.tile([128, 1 + Dmax, HEADS], dt.float32,
                                   tag="e1")
                    nc.vector.tensor_tensor(
                        e1[:, 0:dtot, 0:Hl], hgf[:, 0:dtot, AOFF:AOFF + Hl],
                        adst_l[buf][:, t, 0:Hl].unsqueeze(1).to_broadcast(
                            (128, dtot, Hl)), ALU.add)
                    nc.vector.scalar_tensor_tensor(
                        e1[:, 0:dtot, 0:Hl], e1[:, 0:dtot, 0:Hl], 0.2,
                        e1[:, 0:dtot, 0:Hl], ALU.mult, ALU.max)
                    m = wk3.tile([128, 1 + Dmax, Wm], dt.bfloat16, tag="m")
                    nc.scalar.activation(m[:, 0:dtot, Cl:Cl + Hl],
                                         e1[:, 0:dtot, 0:Hl], AF.Exp)
                    nc.vector.tensor_tensor(
                        m[:, 0:dtot, 0:Cl].rearrange(
                            "p d (h c) -> p d h c", h=Hl),
                        m[:, 0:dtot, Cl:Cl + Hl].unsqueeze(-1).to_broadcast(
                            (128, dtot, Hl, hidl)),
                        hg[:, 0:dtot, 0:Cl].rearrange(
                            "p d (h c) -> p d h c", h=Hl),
                        ALU.mult)

                    spans = []
                    j = 0
                    while j < dtot:
                        span = min(4, dtot - j)
                        spans.append((j, span))
                        j += span
                    spans.sort(key=lambda x: -x[1])
                    Qn = spans[0][1]
                    psA = ps.tile([128, 4 * Wm], dt.float32, tag="psA")
                    for qi, (j, span) in enumerate(spans):
                        nc.tensor.matmul(
                            psA[:, 0:span * Wm], identb[:],
                            m[:, j:j + span, :].rearrange("p a b -> p (a b)"),
                            start=(qi == 0), stop=(qi == len(spans) - 1))

                    sfin = work.tile([128, Wm], dt.float32, tag="sfin")
                    nc.vector.tensor_reduce(
                        sfin[:],
                        psA[:, 0:Qn * Wm].rearrange("p (q w) -> p w q", q=Qn),
                        mybir.AxisListType.X, ALU.add)

                    rs_t = work.tile([128, Hl], dt.float32, tag="rs_t")
                    nc.vector.reciprocal(rs_t[:], sfin[:, Cl:Cl + Hl])
                    out_t = work.tile([128, Cl], dt.float32, tag="out_t")
                    nc.vector.tensor_tensor(
                        out_t[:].rearrange("p (h c) -> p h c", h=Hl),
                        sfin[:, 0:Cl].rearrange("p (h c) -> p h c", h=Hl),
                        rs_t[:].unsqueeze(-1).to_broadcast((128, Hl, hidl)),
                        ALU.mult)
                    nc.vector.tensor_tensor(out_t[:], out_t[:], b_ap(l),
                                            ALU.add)
                    if l < 3:
                        ex = work.tile([128, Cl], dt.float32, tag="ex")
                        nc.scalar.activation(ex[:], out_t[:], AF.Exp)
                        nc.vector.tensor_scalar(
                            ex[:], ex[:], 1.0, -1.0, ALU.min, ALU.add)
                        t2 = work.tile([128, Cl], dt.float32, tag="t2")
                        nc.vector.tensor_scalar(
                            t2[:], out_t[:], 0.0, None, ALU.max)
                        nc.vector.tensor_tensor(
                            f_sb[:, t, :], ex[:], t2[:], ALU.add)
                        # fused dense for layer l+1 on this tile
                        dense_tile(l + 1, t, f_sb[:, t, :])
                    else:
                        oh = work.tile([128, 128], dt.float32, tag="oh")
                        nc.vector.tensor_tensor(
                            oh[:], iotag[:],
                            brel[:, t:t + 1].to_broadcast((128, 128)),
                            ALU.is_equal)
                        nc.tensor.matmul(ps_pool[:], oh[:], out_t[:],
                                         start=(t == 0),
                                         stop=(t == NTILES - 1))

                if l < 3:
                    tc.strict_bb_all_engine_barrier()
                    nc.gpsimd.collective_compute(
                        "AllGather", mybir.AluOpType.bypass,
                        replica_groups=[list(range(NC))],
                        ins=[staging.opt()], outs=[G[l + 1].opt()])
                    tc.strict_bb_all_engine_barrier()

            # ---------- pooling + final linear ----------
            pool_sb = work.tile([128, OUT_C], dt.float32, tag="pool_sb")
            nc.vector.tensor_copy(pool_sb[:], ps_pool[:])
            nc.gpsimd.dma_start(pool_in[:], pool_sb[:])
            tc.strict_bb_all_engine_barrier()
            nc.gpsimd.collective_compute(
                "AllReduce", mybir.AluOpType.add,
                replica_groups=[list(range(NC))],
                ins=[pool_in.opt()], outs=[pool_out.opt()])
            tc.strict_bb_all_engine_barrier()
            psum_sb = work.tile([128, OUT_C], dt.float32, tag="psum_sb")
            nc.gpsimd.dma_start(psum_sb[:], pool_out[:])
            nc.vector.tensor_scalar(psum_sb[:], psum_sb[:], invc[:], None,
                                    ALU.mult)
            psT2 = ps.tile([F_IN, 128], dt.float32, tag="psT")
            nc.tensor.transpose(psT2[0:OUT_C, :], psum_sb[:], identf[:])
            pT = work.tile([OUT_C, 128], dt.float32, tag="pT")
            nc.vector.tensor_copy(pT[:], psT2[0:OUT_C, :])
            ps_out = pspool.tile([128, 2], dt.float32, tag="psO")
            nc.tensor.matmul(ps_out[:], pT[:], wl_sb[:], start=True, stop=True)
            fin_sb = work.tile([128, 2], dt.float32, tag="fin_sb")
            nc.vector.tensor_tensor(fin_sb[:], ps_out[:], bl_ap, ALU.add)
            nc.sync.dma_start(t_out.ap(), fin_sb[:])

    nc.compile()
    return nc


# ----------------------------------------------------------------------
# entry point
# ----------------------------------------------------------------------

def kernel(x, edge_index, batch, W0, as0, ad0, b0, W1, as1, ad1, b1,
           W2, as2, ad2, b2, Wf, asf, adf, bf, Wl, bl):
    from concourse import bass_utils

    pp = _preprocess(edge_index, batch)
    idx_cores = [_build_idx_core(pp, c) for c in range(NC)]
    _, tot_idx = _idx_offsets(pp)
    assert idx_cores[0].shape[1] * 16 == tot_idx

    nc = _build_program(pp, tot_idx)

    x = np.asarray(x, np.float32)
    a_s = [np.asarray(a, np.float32) for a in (as0, as1, as2, asf)]
    a_d = [np.asarray(a, np.float32) for a in (ad0, ad1, ad2, adf)]
    b_l = [np.asarray(a, np.float32) for a in (b0, b1, b2, bf)]
    weights = [np.asarray(W, np.float32) for W in (W0, W1, W2, Wf)]

    # packed broadcast row (must match w_offs layout in _build_program)
    wrow = []
    for l in range(4):
        Cl = OUT_C if l == 3 else C_HID
        wrow += [a_s[l].reshape(Cl), a_d[l].reshape(Cl), b_l[l].reshape(Cl)]
    wrow.append(np.asarray(bl, np.float32).reshape(2))
    wrow = np.concatenate(wrow).reshape(1, -1).astype(np.float32)

    ident_b = np.eye(128, dtype=BF16)
    ident_f = np.eye(128, dtype=np.float32)
    iotag = np.tile(np.arange(128, dtype=np.float32)[None, :], (128, 1))
    zneg = np.zeros((2, 128), BF16)
    # NEG row: asrc field = NEG_VAL at every per-layer offset (32 and 16)
    zv = zneg.view(np.float32)
    zv[1, 32:40] = NEG_VAL   # layers 0-2: asrc field, all 8 heads
    zv[1, 16:17] = NEG_VAL   # final layer: asf field (1 head)
    invc = (1.0 / np.maximum(pp["cnt"], 1.0)).reshape(G_GRAPHS, 1)

    in_maps = []
    for c in range(NC):
        xp = np.zeros((NPAD, F_IN), BF16)
        xp[:NLOC] = x[pp["perm"][c]].astype(BF16)
        im = dict(
            x=xp, idx=idx_cores[c], wrow=wrow,
            identb=ident_b, identf=ident_f, iotag=iotag,
            brel=pp["batchrel"][c].reshape(NTILES, 128).T.astype(np.float32),
            zneg=zneg, invc=invc.astype(np.float32),
            Wl=np.asarray(Wl, np.float32))
        for l in range(4):
            im[f"W{l}"] = weights[l]
        in_maps.append(im)

    res = bass_utils.run_bass_kernel_spmd(nc, in_maps,
                                          core_ids=list(range(NC)))
    kernel.last_results = res
    kernel.last_nc = nc
    kernel.last_in_maps = in_maps
    return res.results[0]["out"]


# revision 21
# speedup vs baseline: 1.0189x; 1.0189x over previous
"""GAT (4-layer, 8-head) message-passing kernel for 8 Trainium2 NeuronCores.

Strategy (self-contained; shapes hardcoded from the problem spec):
  - Nodes are sharded by destination across 8 cores (6250 nodes each);
    within each core nodes are permuted by (degree, lo-hi balance) so that
    128-node tiles have near-uniform in-degree.
  - Per layer: each core computes hW = f @ W for its local nodes (PE),
    per-node attention coefficients asrc/adst (DVE), packs [bf16 h | f32
    asrc] into 256-byte table rows, and AllGathers the table to all cores.
  - Per-edge work uses dma_gather (256B rows) with int16 indices. The
    int16 range limit is handled with two gather bases (row 0 and row
    3*SHARD); sources from ranks 3-4 are reachable from either base, which
    lets per-node slot assignment meet uniform per-tile column cuts with
    only a narrow double-gathered strip.
  - The appended self-loop of every node is NOT gathered: slot column 0 is
    filled by an on-chip copy of the local staging tile.
  - Slots land [node-on-partition, column]; softmax (no max-subtraction
    needed, logits are O(10)) via ACT Exp; the segment sum (numerator and
    denominator in one shot) is identity-lhsT PE matmuls accumulating
    columns into PSUM.
  - The dense phase of layer l+1 is fused into the edge loop of layer l so
    PE/DVE work hides under the gather descriptor generation (GpSimd).
  - global_mean_pool: per-core onehot matmul accumulation + AllReduce,
    then the final 32->2 linear on every core (identical outputs).
"""
import sys
sys.path.insert(0, "/opt/trn_rl_repo")

import numpy as np
import ml_dtypes

BF16 = ml_dtypes.bfloat16

N = 50000
E = 1600000
G_GRAPHS = 128
F_IN = 128
HEADS, HID = 8, 8
C_HID = 64
OUT_C = 32
NC = 8
NLOC = N // NC            # 6250
SHARD = NLOC + 2          # 6252 rows per rank in the table (+Z +NEG)
NTILES = 49
NPAD = NTILES * 128
BASE1 = 7 * SHARD + NLOC + 1 - 32767   # 17248: widest dual-reach zone
NEG_VAL = -1e30
Z_OFF, NEG_OFF = NLOC, NLOC + 1                    # base0-relative (rank 0)
Z_OFF1 = 3 * SHARD + NLOC - BASE1                  # base1-relative (rank 3)
NEG_OFF1 = Z_OFF1 + 1


# ----------------------------------------------------------------------
# CPU preprocessing
# ----------------------------------------------------------------------

def _preprocess(edge_index, batch):
    import time as _time
    # No self-loop append: the appended loops are handled on-chip (col 0).
    src = np.asarray(edge_index[0]).astype(np.int64)
    dst = np.asarray(edge_index[1]).astype(np.int64)
    batch = np.asarray(batch).astype(np.int64)
    deg = np.bincount(dst, minlength=N)
    owner = np.repeat(np.arange(NC), NLOC)

    # classes are row-granular: lo = base0-only, mid = dual-reach, hi = base1-
    # only. Rows depend on positions, so seed with a deg-only sort, derive
    # classes, then re-sort with the balance secondary key.
    order_pos = np.empty(N, np.int64)
    perm = []
    for c in range(NC):
        local = np.arange(c * NLOC, (c + 1) * NLOC)
        p = local[np.argsort(-deg[local], kind="stable")]
        perm.append(p)
        order_pos[p] = np.arange(NLOC)

    def node_classes():
        row = owner * SHARD + order_pos
        return np.where(row[src] < BASE1, 0,
                        np.where(row[src] <= 32767, 1, 2))

    cls = node_classes()
    nlow_of = np.bincount(dst, weights=(cls == 0), minlength=N).astype(np.int64)
    nhigh_of = np.bincount(dst, weights=(cls == 2), minlength=N).astype(np.int64)
    for c in range(NC):
        local = np.arange(c * NLOC, (c + 1) * NLOC)
        key = deg[local] * 10000 + (nlow_of[local] - nhigh_of[local]) * 30
        p = local[np.argsort(-key, kind="stable")]
        perm[c] = p
        order_pos[p] = np.arange(NLOC)
    cls = node_classes()
    nlow_of = np.bincount(dst, weights=(cls == 0), minlength=N).astype(np.int64)
    nhigh_of = np.bincount(dst, weights=(cls == 2), minlength=N).astype(np.int64)

    # Local search: within-core swaps reducing the global per-tile objective
    # sum(Dmax + max(0, Lmax+Hmax-Dmax)). Swaps in ranks 2/5 must not cross
    # the class row boundaries (classes above are frozen).
    bound2 = BASE1 - 2 * SHARD
    bound5 = 32767 - 5 * SHARD
    t0 = _time.time()

    def tile_stats(t):
        s, e = t * 128, min(t * 128 + 128, NLOC)
        dm = lm = hm = 0
        for c in range(NC):
            p = perm[c][s:e]
            dm = max(dm, int(deg[p].max()))
            lm = max(lm, int(nlow_of[p].max()))
            hm = max(hm, int(nhigh_of[p].max()))
        return [dm, lm, hm]

    def tcost(st):
        return st[0] + max(0, st[1] + st[2] - st[0])

    stats = [tile_stats(t) for t in range(NTILES)]
    for _pass in range(4):
        improved = False
        for t in range(NTILES):
            s, e = t * 128, min(t * 128 + 128, NLOC)
            for t2 in range(max(0, t - 4), min(NTILES, t + 5)):
                if t2 == t:
                    continue
                s2, e2 = t2 * 128, min(t2 * 128 + 128, NLOC)
                c0 = tcost(stats[t]) + tcost(stats[t2])
                for c in range(NC):
                    p = perm[c]
                    win, win2 = p[s:e], p[s2:e2]
                    for arr in (nlow_of, nhigh_of, deg):
                        i = s + int(np.argmax(arr[win]))
                        for j in (s2 + int(np.argmin(arr[win2])),
                                  s2 + int(np.argmax(deg[win2]))):
                            if c == 2 and (i >= bound2) != (j >= bound2):
                                continue
                            if c == 5 and (i <= bound5) != (j <= bound5):
                                continue
                            p[i], p[j] = p[j], p[i]
                            st1, st2 = tile_stats(t), tile_stats(t2)
                            if tcost(st1) + tcost(st2) < c0:
                                stats[t], stats[t2] = st1, st2
                                c0 = tcost(st1) + tcost(st2)
                                improved = True
                            else:
                                p[i], p[j] = p[j], p[i]
        if not improved or _time.time() - t0 > 20:
            break
    for c in range(NC):
        order_pos[perm[c]] = np.arange(NLOC)
    row_of = np.empty(N, np.int64)
    for c in range(NC):
        row_of[perm[c]] = c * SHARD + order_pos[perm[c]]

    core_e = dst // NLOC
    pos_e = order_pos[dst]
    srow = row_of[src]

    ndeg = np.zeros((NC, NPAD), np.int32)
    nlow = np.zeros((NC, NPAD), np.int32)
    nmid = np.zeros((NC, NPAD), np.int32)
    np.add.at(ndeg, (core_e, pos_e), 1)
    np.add.at(nlow, (core_e, pos_e), (cls == 0).astype(np.int32))
    np.add.at(nmid, (core_e, pos_e), (cls == 1).astype(np.int32))
    nhigh = ndeg - nlow - nmid

    # Per-tile gather geometry: D = max degree (gathered cols, excl. self),
    # K = base0-only col count, W = double-gathered strip width.
    D = np.zeros(NTILES, np.int32)
    K = np.zeros(NTILES, np.int32)
    Wd = np.zeros(NTILES, np.int32)
    for t in range(NTILES):
        s = t * 128
        real = min(128, NLOC - s) if s < NLOC else 0
        if real:
            dm = int(ndeg[:, s:s + real].max())
            Lmax = int(nlow[:, s:s + real].max())
            Hmax = int(nhigh[:, s:s + real].max())
        else:
            dm = Lmax = Hmax = 0
        w = max(0, Lmax + Hmax - dm)
        k = dm - Hmax
        D[t], K[t], Wd[t] = dm, k, w

    # Slot assignment: lo+mid left-fill from slot 0, hi right-aligned at D.
    key2 = ((core_e * NPAD + pos_e) * 4 + cls)
    o = np.argsort(key2, kind="stable")
    srow_s, core_s, pos_s, cls_s = srow[o], core_e[o], pos_e[o], cls[o]
    gid = core_s * NPAD + pos_s
    first = np.r_[True, gid[1:] != gid[:-1]]
    cum = np.arange(len(gid)) - np.maximum.accumulate(
        np.where(first, np.arange(len(gid)), 0))
    t_of = pos_s // 128
    col = np.where(cls_s < 2, cum,
                   D[t_of] - nhigh[core_s, pos_s] + cum
                   - nlow[core_s, pos_s] - nmid[core_s, pos_s])
    Dmax = int(D.max())
    which = np.full((NC, NPAD, Dmax), 4, np.int8)   # 4 = pad
    rowg = np.zeros((NC, NPAD, Dmax), np.int64)
    which[core_s, pos_s, col] = cls_s
    rowg[core_s, pos_s, col] = srow_s

    batchrel = np.full((NC, NPAD), 255, np.int64)
    for c in range(NC):
        batchrel[c, :NLOC] = batch[perm[c]]
    cnt = np.bincount(batch, minlength=G_GRAPHS).astype(np.float32)
    return dict(perm=perm, which=which, rowg=rowg, D=D, K=K, Wd=Wd,
                batchrel=batchrel, cnt=cnt)


def _build_idx_core(pp, c):
    """Linear int16 idx stream for core c: per tile, calls P0=[X|A], P1=[Y|B]."""
    chunks = []
    for t in range(NTILES):
        d, k, w = int(pp["D"][t]), int(pp["K"][t]), int(pp["Wd"][t])
        wh = pp["which"][c, t * 128:(t + 1) * 128, :d]
        rg = pp["rowg"][c, t * 128:(t + 1) * 128, :d]
        rel0 = rg
        rel1 = rg - BASE1
        # P0 from base0: X cols [0,k): lo/mid -> row, pad -> NEG
        #                A cols [k,k+w): lo/mid -> row, hi -> Z, pad -> NEG
        wx = wh[:, :k]
        iX = np.where(wx <= 1, rel0[:, :k], NEG_OFF)      # cls2 impossible in X
        wa = wh[:, k:k + w]
        iA = np.where(wa <= 1, rel0[:, k:k + w],
                      np.where(wa == 2, Z_OFF, NEG_OFF))
        # P1 from base1: Y cols [k+w,d): mid/hi -> row-BASE1, pad -> NEG
        #                B cols [k,k+w): hi -> row-BASE1, else -> Z
        wy = wh[:, k + w:]
        iY = np.where((wy == 1) | (wy == 2), rel1[:, k + w:], NEG_OFF1)
        iB = np.where(wa == 2, rel1[:, k:k + w], Z_OFF1)
        for arr in (iX, iA, iY, iB):
            # linearize: for col j, node i -> stream slot j*128+i
            chunks.append(arr.T.reshape(-1))
    lin = np.concatenate(chunks).astype(np.int16)
    assert lin.min() >= 0
    return lin.reshape(-1, 16).T.copy()    # [16, TOT/16]


def _idx_offsets(pp):
    """Column offsets (in idx units /16) of each call, per tile."""
    offs = []
    cur = 0
    for t in range(NTILES):
        d, k, w = int(pp["D"][t]), int(pp["K"][t]), int(pp["Wd"][t])
        o = {}
        for name, cols in (("P0", k + w), ("P1", d - k)):
            o[name] = (cur // 16, cols * 128)
            cur += cols * 128
        offs.append(o)
    return offs, cur


# ----------------------------------------------------------------------
# Bass program
# ----------------------------------------------------------------------

def _build_program(pp, tot_idx):
    import concourse.bacc as bacc
    import concourse.mybir as mybir
    import concourse.tile as tile
    from concourse.library_config import mlp

    dt = mybir.dt
    AF = mybir.ActivationFunctionType
    ALU = mybir.AluOpType

    D, K, Wd = pp["D"], pp["K"], pp["Wd"]
    Dmax = int(D.max())
    Wmax = int(Wd.max())
    offs, _ = _idx_offsets(pp)

    nc = bacc.Bacc("TRN2", target_bir_lowering=False, debug=False,
                   num_devices=NC)

    t_x = nc.dram_tensor("x", [NPAD, F_IN], dt.bfloat16, kind="ExternalInput")
    t_idx = nc.dram_tensor("idx", [16, tot_idx // 16], dt.int16,
                           kind="ExternalInput")
    t_w = [nc.dram_tensor(f"W{l}", [F_IN if l == 0 else C_HID,
                                    OUT_C if l == 3 else C_HID],
                          dt.float32, kind="ExternalInput") for l in range(4)]
    # packed broadcast row: per layer AS|AD|B (Cl each), then bl (2)
    w_offs = {}
    cur = 0
    for l in range(4):
        Cl = OUT_C if l == 3 else C_HID
        w_offs[f"AS{l}"] = cur; cur += Cl
        w_offs[f"AD{l}"] = cur; cur += Cl
        w_offs[f"B{l}"] = cur; cur += Cl
    w_offs["bl"] = cur; cur += 2
    WROW = cur                                           # 674
    t_wrow = nc.dram_tensor("wrow", [1, WROW], dt.float32,
                            kind="ExternalInput")
    t_identb = nc.dram_tensor("identb", [128, 128], dt.bfloat16,
                              kind="ExternalInput")
    t_identf = nc.dram_tensor("identf", [128, 128], dt.float32,
                              kind="ExternalInput")
    t_iotag = nc.dram_tensor("iotag", [128, 128], dt.float32,
                             kind="ExternalInput")
    t_brel = nc.dram_tensor("brel", [128, NTILES], dt.float32,
                            kind="ExternalInput")
    t_zneg = nc.dram_tensor("zneg", [2, 128], dt.bfloat16,
                            kind="ExternalInput")
    t_invc = nc.dram_tensor("invc", [128, 1], dt.float32,
                            kind="ExternalInput")
    t_wl = nc.dram_tensor("Wl", [OUT_C, 2], dt.float32, kind="ExternalInput")
    t_out = nc.dram_tensor("out", [G_GRAPHS, 2], dt.float32,
                           kind="ExternalOutput")

    with tile.TileContext(nc) as tc:
        with tc.tile_pool(name="res", bufs=1) as res, \
             tc.tile_pool(name="work", bufs=2) as work, \
             tc.tile_pool(name="wk3", bufs=3) as wk3, \
             tc.tile_pool(name="ps", bufs=2, space="PSUM") as ps, \
             tc.tile_pool(name="pspool", bufs=1, space="PSUM") as pspool, \
             tc.tile_pool(name="dram", bufs=1, space="DRAM") as dram:

            nc.gpsimd.load_library(mlp)

            # ---- resident loads ----
            idx_sb = res.tile([128, tot_idx // 16], dt.int16)
            for g in range(8):
                nc.sync.dma_start(idx_sb[16 * g:16 * g + 16, :], t_idx.ap())
            identb = res.tile([128, 128], dt.bfloat16)
            nc.sync.dma_start(identb[:], t_identb.ap())
            identf = res.tile([128, 128], dt.float32)
            nc.sync.dma_start(identf[:], t_identf.ap())
            iotag = res.tile([128, 128], dt.float32)
            nc.sync.dma_start(iotag[:], t_iotag.ap())
            brel = res.tile([128, NTILES], dt.float32)
            nc.sync.dma_start(brel[:], t_brel.ap())
            zneg = res.tile([2, 128], dt.bfloat16)
            nc.sync.dma_start(zneg[:], t_zneg.ap())
            invc = res.tile([128, 1], dt.float32)
            nc.sync.dma_start(invc[:], t_invc.ap())
            wl_sb = res.tile([OUT_C, 2], dt.float32)
            nc.sync.dma_start(wl_sb[:], t_wl.ap())
            w_sb = []
            for l in range(4):
                Fl = F_IN if l == 0 else C_HID
                Cl = OUT_C if l == 3 else C_HID
                w_sb.append(res.tile([Fl, Cl], dt.float32, tag=f"w{l}",
                                     name=f"w{l}"))
                nc.sync.dma_start(w_sb[l][:], t_w[l].ap())

            # broadcast the packed weight row to all 128 partitions via PE
            wrow_sb = res.tile([1, WROW], dt.float32)
            nc.sync.dma_start(wrow_sb[:], t_wrow.ap())
            ones_sb = res.tile([1, 128], dt.float32)
            nc.vector.memset(ones_sb[:], 1.0)
            bcast = res.tile([128, WROW], dt.float32)
            half = 288
            for o0 in range(0, WROW, half):
                o1 = min(o0 + half, WROW)
                psB = ps.tile([128, half], dt.float32, tag="psA")
                nc.tensor.matmul(psB[:, 0:o1 - o0], ones_sb[:],
                                 wrow_sb[:, o0:o1], start=True, stop=True)
                nc.vector.tensor_copy(bcast[:, o0:o1], psB[:, 0:o1 - o0])

            def as_ap(l):
                Cl = OUT_C if l == 3 else C_HID
                return bcast[:, w_offs[f"AS{l}"]:w_offs[f"AS{l}"] + Cl]

            def ad_ap(l):
                Cl = OUT_C if l == 3 else C_HID
                return bcast[:, w_offs[f"AD{l}"]:w_offs[f"AD{l}"] + Cl]

            def b_ap(l):
                Cl = OUT_C if l == 3 else C_HID
                return bcast[:, w_offs[f"B{l}"]:w_offs[f"B{l}"] + Cl]

            bl_ap = bcast[:, w_offs["bl"]:w_offs["bl"] + 2]

            f_sb = res.tile([128, NTILES, C_HID], dt.float32)
            # ping-pong staging copies (current layer / next layer)
            stg_sb = [res.tile([128, NTILES, 128], dt.bfloat16, tag=f"stg{i}",
                               name=f"stg{i}") for i in range(2)]
            adst_l = [res.tile([128, NTILES, HEADS], dt.float32,
                               tag=f"adst{i}", name=f"adst{i}")
                      for i in range(2)]
            staging = dram.tile([SHARD, 128], dt.bfloat16)

            pool_in = dram.tile([G_GRAPHS, OUT_C], dt.float32)
            pool_out = dram.tile([G_GRAPHS, OUT_C], dt.float32,
                                 addr_space="Shared")
            ps_pool = pspool.tile([128, OUT_C], dt.float32)

            G = [dram.tile([NC * SHARD, 128], dt.bfloat16,
                           addr_space="Shared", tag=f"G{l}", name=f"G{l}")
                 for l in range(4)]

            def dense_tile(l, t, fin):
                """Compute layer-l hW/asrc/adst for tile t from features fin;
                write staging row + stg_sb/adst_l for layer l."""
                Cl = OUT_C if l == 3 else C_HID
                Hl = 1 if l == 3 else HEADS
                AOFF = Cl // 2
                buf = l % 2
                Fl = F_IN if l == 0 else C_HID
                fT = work.tile([F_IN, 128], dt.float32, tag="fT")
                psT = ps.tile([F_IN, 128], dt.float32, tag="psT")
                if l == 0:
                    psTb = psT[:].bitcast(dt.bfloat16)[:, 0:128]
                    nc.tensor.transpose(psTb, fin, identb[:])
                    nc.scalar.activation(fT[:], psTb, AF.Copy)
                else:
                    nc.tensor.transpose(psT[0:C_HID, :], fin, identf[:])
                    nc.scalar.activation(fT[0:Fl, :], psT[0:Fl, :], AF.Copy)
                psH = ps.tile([128, Cl], dt.float32, tag="psH")
                nc.tensor.matmul(psH[:], fT[0:Fl, :], w_sb[l][:],
                                 start=True, stop=True)
                hw = work.tile([128, C_HID], dt.float32, tag="hw")
                nc.vector.tensor_copy(hw[:, 0:Cl], psH[:])

                tmp = work.tile([128, C_HID], dt.float32, tag="tmp")
                nc.vector.tensor_tensor(tmp[:, 0:Cl], hw[:, 0:Cl], as_ap(l),
                                        ALU.mult)
                asrc_t = work.tile([128, HEADS], dt.float32, tag="asrc_t")
                nc.vector.tensor_reduce(
                    asrc_t[:, 0:Hl],
                    tmp[:, 0:Cl].rearrange("p (h c) -> p h c", h=Hl),
                    mybir.AxisListType.X, ALU.add)
                nc.vector.tensor_tensor(tmp[:, 0:Cl], hw[:, 0:Cl], ad_ap(l),
                                        ALU.mult)
                nc.vector.tensor_reduce(
                    adst_l[buf][:, t, 0:Hl],
                    tmp[:, 0:Cl].rearrange("p (h c) -> p h c", h=Hl),
                    mybir.AxisListType.X, ALU.add)

                stg = work.tile([128, 128], dt.bfloat16, tag="stg")
                nc.vector.memset(stg[:, Cl:128], 0)
                nc.scalar.activation(stg[:, 0:Cl], hw[:, 0:Cl], AF.Copy)
                stgf = stg[:].bitcast(dt.float32)
                nc.vector.tensor_copy(stgf[:, AOFF:AOFF + Hl],
                                      asrc_t[:, 0:Hl])
                # persist for the self-loop slot + ship to DRAM for AllGather
                nc.vector.tensor_copy(stg_sb[buf][:, t, :], stg[:])
                rows = min(128, NLOC - t * 128)
                nc.sync.dma_start(
                    staging[t * 128:t * 128 + rows, :], stg[0:rows, :])

            # ---------- layer-0 dense (batched: wide vector ops) ----------
            ft_all = res.tile([128, NTILES, F_IN], dt.bfloat16)
            for t in range(NTILES):
                nc.sync.dma_start(ft_all[:, t, :],
                                  t_x.ap()[t * 128:(t + 1) * 128, :])
            hw_all = res.tile([128, NTILES, C_HID], dt.float32)
            for t in range(NTILES):
                psT0 = ps.tile([F_IN, 128], dt.float32, tag="psT")
                psTb = psT0[:].bitcast(dt.bfloat16)[:, 0:128]
                nc.tensor.transpose(psTb, ft_all[:, t, :], identb[:])
                fT0 = work.tile([F_IN, 128], dt.float32, tag="fT")
                nc.scalar.activation(fT0[:], psTb, AF.Copy)
                psH0 = ps.tile([128, C_HID], dt.float32, tag="psH")
                nc.tensor.matmul(psH0[:], fT0[:], w_sb[0][:],
                                 start=True, stop=True)
                nc.vector.tensor_copy(hw_all[:, t, :], psH0[:])
            tmp_all = res.tile([128, NTILES, C_HID], dt.float32)
            nc.vector.tensor_tensor(
                tmp_all[:], hw_all[:],
                as_ap(0).unsqueeze(1).to_broadcast((128, NTILES, C_HID)),
                ALU.mult)
            asrc_all = res.tile([128, NTILES, HEADS], dt.float32)
            nc.vector.tensor_reduce(
                asrc_all[:],
                tmp_all[:].rearrange("p t (h c) -> p t h c", h=HEADS),
                mybir.AxisListType.X, ALU.add)
            nc.vector.tensor_tensor(
                tmp_all[:], hw_all[:],
                ad_ap(0).unsqueeze(1).to_broadcast((128, NTILES, C_HID)),
                ALU.mult)
            nc.vector.tensor_reduce(
                adst_l[0][:],
                tmp_all[:].rearrange("p t (h c) -> p t h c", h=HEADS),
                mybir.AxisListType.X, ALU.add)
            nc.vector.memset(stg_sb[0][:], 0)
            nc.scalar.activation(stg_sb[0][:, :, 0:C_HID], hw_all[:], AF.Copy)
            stgf_all = stg_sb[0][:].bitcast(dt.float32)
            nc.vector.tensor_copy(stgf_all[:, :, 32:40], asrc_all[:])
            for t in range(NTILES):
                rows = min(128, NLOC - t * 128)
                nc.sync.dma_start(staging[t * 128:t * 128 + rows, :],
                                  stg_sb[0][0:rows, t, :])
            nc.sync.dma_start(staging[NLOC:NLOC + 2, :], zneg[:])
            tc.strict_bb_all_engine_barrier()
            nc.gpsimd.collective_compute(
                "AllGather", mybir.AluOpType.bypass,
                replica_groups=[list(range(NC))],
                ins=[staging.opt()], outs=[G[0].opt()])
            tc.strict_bb_all_engine_barrier()

            # ---------- per-layer edge phase (dense of l+1 fused in) ----------
            for l in range(4):
                Cl = OUT_C if l == 3 else C_HID
                Hl = 1 if l == 3 else HEADS
                hidl = OUT_C if l == 3 else HID
                Wm = Cl + Hl
                AOFF = Cl // 2
                buf = l % 2
                Gap = G[l][:]
                G1ap = G[l][:][BASE1:NC * SHARD, :]

                for t in range(NTILES):
                    d, k, w = int(D[t]), int(K[t]), int(Wd[t])
                    dtot = 1 + d
                    hg = wk3.tile([128, 1 + Dmax + Wmax, 128], dt.bfloat16,
                                  tag="hg")
                    # self-loop slot: on-chip copy of the local staging tile
                    nc.scalar.activation(hg[:, 0, :], stg_sb[buf][:, t, :],
                                         AF.Copy)
                    o = offs[t]
                    if k + w:
                        c0, n = o["P0"]
                        nc.gpsimd.dma_gather(
                            hg[:, 1:1 + k + w, :], Gap,
                            idx_sb[:, c0:c0 + n // 16], n, n, 128,
                            single_packet=False)
                    if d - k:
                        c0, n = o["P1"]
                        nc.gpsimd.dma_gather(
                            hg[:, 1 + k + w:1 + d + w, :], G1ap,
                            idx_sb[:, c0:c0 + n // 16], n, n, 128,
                            single_packet=False)
                    if w:
                        nc.vector.tensor_tensor(
                            hg[:, 1 + k:1 + k + w, 0:Cl],
                            hg[:, 1 + k:1 + k + w, 0:Cl],
                            hg[:, 1 + d:1 + d + w, 0:Cl], ALU.add)
                        hgf = hg[:].bitcast(dt.float32)
                        nc.vector.tensor_tensor(
                            hgf[:, 1 + k:1 + k + w, AOFF:AOFF + Hl],
                            hgf[:, 1 + k:1 + k + w, AOFF:AOFF + Hl],
                            hgf[:, 1 + d:1 + d + w, AOFF:AOFF + Hl], ALU.add)

                    hgf = hg[:].bitcast(dt.float32)
                    e1 = work.tile([128, 1 + Dmax, HEADS], dt.float32,
                                   tag="e1")
                    nc.vector.tensor_tensor(
                        e1[:, 0:dtot, 0:Hl], hgf[:, 0:dtot, AOFF:AOFF + Hl],
                        adst_l[buf][:, t, 0:Hl].unsqueeze(1).to_broadcast(
                            (128, dtot, Hl)), ALU.add)
                    nc.vector.scalar_tensor_tensor(
                        e1[:, 0:dtot, 0:Hl], e1[:, 0:dtot, 0:Hl], 0.2,
                        e1[:, 0:dtot, 0:Hl], ALU.mult, ALU.max)
                    m = wk3.tile([128, 1 + Dmax, Wm], dt.bfloat16, tag="m")
                    nc.scalar.activation(m[:, 0:dtot, Cl:Cl + Hl],
                                         e1[:, 0:dtot, 0:Hl], AF.Exp)
                    nc.vector.tensor_tensor(
                        m[:, 0:dtot, 0:Cl].rearrange(
                            "p d (h c) -> p d h c", h=Hl),
                        m[:, 0:dtot, Cl:Cl + Hl].unsqueeze(-1).to_broadcast(
                            (128, dtot, Hl, hidl)),
                        hg[:, 0:dtot, 0:Cl].rearrange(
                            "p d (h c) -> p d h c", h=Hl),
                        ALU.mult)

                    spans = []
                    j = 0
                    while j < dtot:
                        span = min(4, dtot - j)
                        spans.append((j, span))
                        j += span
                    spans.sort(key=lambda x: -x[1])
                    Qn = spans[0][1]
                    psA = ps.tile([128, 4 * Wm], dt.float32, tag="psA")
                    for qi, (j, span) in enumerate(spans):
                        nc.tensor.matmul(
                            psA[:, 0:span * Wm], identb[:],
                            m[:, j:j + span, :].rearrange("p a b -> p (a b)"),
                            start=(qi == 0), stop=(qi == len(spans) - 1))

                    sfin = work.tile([128, Wm], dt.float32, tag="sfin")
                    nc.vector.tensor_reduce(
                        sfin[:],
                        psA[:, 0:Qn * Wm].rearrange("p (q w) -> p w q", q=Qn),
                        mybir.AxisListType.X, ALU.add)

                    rs_t = work.tile([128, Hl], dt.float32, tag="rs_t")
                    nc.vector.reciprocal(rs_t[:], sfin[:, Cl:Cl + Hl])
                    out_t = work.tile([128, Cl], dt.float32, tag="out_t")
                    nc.vector.tensor_tensor(
                        out_t[:].rearrange("p (h c) -> p h c", h=Hl),
                        sfin[:, 0:Cl].rearrange("p (h c) -> p h c", h=Hl),
                        rs_t[:].unsqueeze(-1).to_broadcast((128, Hl, hidl)),
                        ALU.mult)
                    nc.vector.tensor_tensor(out_t[:], out_t[:], b_ap(l),
                                            ALU.add)
                    if l < 3:
                        ex = work.tile([128, Cl], dt.float32, tag="ex")
                        nc.scalar.activation(ex[:], out_t[:], AF.Exp)
                        nc.vector.tensor_scalar(
                            ex[:], ex[:], 1.0, -1.0, ALU.min, ALU.add)
                        t2 = work.tile([128, Cl], dt.float32, tag="t2")
                        nc.vector.tensor_scalar(
                            t2[:], out_t[:], 0.0, None, ALU.max)
                        nc.vector.tensor_tensor(
                            f_sb[:, t, :], ex[:], t2[:], ALU.add)
                        # fused dense for layer l+1 on this tile
                        dense_tile(l + 1, t, f_sb[:, t, :])
                    else:
                        oh = work.tile([128, 128], dt.float32, tag="oh")
                        nc.vector.tensor_tensor(
                            oh[:], iotag[:],
                            brel[:, t:t + 1].to_broadcast((128, 128)),
                            ALU.is_equal)
                        nc.tensor.matmul(ps_pool[:], oh[:], out_t[:],
                                         start=(t == 0),
                                         stop=(t == NTILES - 1))

                if l < 3:
                    tc.strict_bb_all_engine_barrier()
                    nc.gpsimd.collective_compute(
                        "AllGather", mybir.AluOpType.bypass,
                        replica_groups=[list(range(NC))],
                        ins=[staging.opt()], outs=[G[l + 1].opt()])
                    tc.strict_bb_all_engine_barrier()

            # ---------- pooling + final linear ----------
            pool_sb = work.tile([128, OUT_C], dt.float32, tag="pool_sb")
            nc.vector.tensor_copy(pool_sb[:], ps_pool[:])
            nc.gpsimd.dma_start(pool_in[:], pool_sb[:])
            tc.strict_bb_all_engine_barrier()
            nc.gpsimd.collective_compute(
                "AllReduce", mybir.AluOpType.add,
                replica_groups=[list(range(NC))],
                ins=[pool_in.opt()], outs=[pool_out.opt()])
            tc.strict_bb_all_engine_barrier()
            psum_sb = work.tile([128, OUT_C], dt.float32, tag="psum_sb")
            nc.gpsimd.dma_start(psum_sb[:], pool_out[:])
            nc.vector.tensor_scalar(psum_sb[:], psum_sb[:], invc[:], None,
                                    ALU.mult)
            psT2 = ps.tile([F_IN, 128], dt.float32, tag="psT")
            nc.tensor.transpose(psT2[0:OUT_C, :], psum_sb[:], identf[:])
            pT = work.tile([OUT_C, 128], dt.float32, tag="pT")
            nc.vector.tensor_copy(pT[:], psT2[0:OUT_C, :])
            ps_out = pspool.tile([128, 2], dt.float32, tag="psO")
            nc.tensor.matmul(ps_out[:], pT[:], wl_sb[:], start=True, stop=True)
            fin_sb = work.tile([128, 2], dt.float32, tag="fin_sb")
            nc.vector.tensor_tensor(fin_sb[:], ps_out[:], bl_ap, ALU.add)
            nc.sync.dma_start(t_out.ap(), fin_sb[:])

    nc.compile()
    return nc


# ----------------------------------------------------------------------
# entry point
# ----------------------------------------------------------------------

def kernel(x, edge_index, batch, W0, as0, ad0, b0, W1, as1, ad1, b1,
           W2, as2, ad2, b2, Wf, asf, adf, bf, Wl, bl):
    from concourse import bass_utils

    pp = _preprocess(edge_index, batch)
    idx_cores = [_build_idx_core(pp, c) for c in range(NC)]
    _, tot_idx = _idx_offsets(pp)
    assert idx_cores[0].shape[1] * 16 == tot_idx

    nc = _build_program(pp, tot_idx)

    x = np.asarray(x, np.float32)
    a_s = [np.asarray(a, np.float32) for a in (as0, as1, as2, asf)]
    a_d = [np.asarray(a, np.float32) for a in (ad0, ad1, ad2, adf)]
    b_l = [np.asarray(a, np.float32) for a in (b0, b1, b2, bf)]
    weights = [np.asarray(W, np.float32) for W in (W0, W1, W2, Wf)]

    # packed broadcast row (must match w_offs layout in _build_program)
    wrow = []
    for l in range(4):
        Cl = OUT_C if l == 3 else C_HID
        wrow += [a_s[l].reshape(Cl), a_d[l].reshape(Cl), b_l[l].reshape(Cl)]
    wrow.append(np.asarray(bl, np.float32).reshape(2))
    wrow = np.concatenate(wrow).reshape(1, -1).astype(np.float32)

    ident_b = np.eye(128, dtype=BF16)
    ident_f = np.eye(128, dtype=np.float32)
    iotag = np.tile(np.arange(128, dtype=np.float32)[None, :], (128, 1))
    zneg = np.zeros((2, 128), BF16)
    # NEG row: asrc field = NEG_VAL at every per-layer offset (32 and 16)
    zv = zneg.view(np.float32)
    zv[1, 32:40] = NEG_VAL   # layers 0-2: asrc field, all 8 heads
    zv[1, 16:17] = NEG_VAL   # final layer: asf field (1 head)
    invc = (1.0 / np.maximum(pp["cnt"], 1.0)).reshape(G_GRAPHS, 1)

    in_maps = []
    for c in range(NC):
        xp = np.zeros((NPAD, F_IN), BF16)
        xp[:NLOC] = x[pp["perm"][c]].astype(BF16)
        im = dict(
            x=xp, idx=idx_cores[c], wrow=wrow,
            identb=ident_b, identf=ident_f, iotag=iotag,
            brel=pp["batchrel"][c].reshape(NTILES, 128).T.astype(np.float32),
            zneg=zneg, invc=invc.astype(np.float32),
            Wl=np.asarray(Wl, np.float32))
        for l in range(4):
            im[f"W{l}"] = weights[l]
        in_maps.append(im)

    res = bass_utils.run_bass_kernel_spmd(nc, in_maps,
                                          core_ids=list(range(NC)))
    kernel.last_results = res
    kernel.last_nc = nc
    kernel.last_in_maps = in_maps
    return res.results[0]["out"]
